# revision 2
# baseline (speedup 1.0000x reference)
"""HaarMSELoss kernel for Trainium2 (8 NeuronCores, data-parallel).

Math: the 2x2 Haar transform used by the reference is (up to the 0.5
scaling) an orthogonal Hadamard transform, so for each 2x2 block
LL^2+LH^2+HL^2+HH^2 == a^2+b^2+c^2+d^2 of the block entries of
(input - target).  Hence

  loss = sum_bands mean((haar(x)-haar(y))^2)
       = sum((x-y)^2) / (B*C*(H/2)*(W/2))

i.e. a pure squared-difference reduction.  Each core reduces 1/8 of the
elements; the host sums the 8x128 per-partition partials (f64) and
divides.

Layout: per core the two chunks are interleaved host-side into one
[128, 2, FREE] array (row p = x-row p, y-row p) so each SBUF tile of
both operands arrives together.

v2 pipeline (raw bass, explicit sems; every wait is a standalone
single-sem wait):
  Each tile is split into two half-width DMAs, one on each HWDGE queue
  (qSyncDynamicHW issued by SP, qScalarDynamicHW issued by ACT), so both
  DMA engines stream concurrently.  Tile widths taper at the end
  (4096 x7, 2048, 1024, 512, 512) so the serial sub+square tail after
  the last byte lands is small.
  SP  : half-tile loads (queue S), final drain waits
  ACT : half-tile loads (queue A) + stats[:,t] = sum(d^2) via
        activation(Square, accum_out) + final stats store
  DVE : d = x - y in place
"""

import numpy as np

_B, _C, _H, _W = 4, 32, 512, 512
_TOTAL = _B * _C * _H * _W          # 33_554_432
_NCORES = 8
_PER_CORE = _TOTAL // _NCORES       # 4_194_304
_P = 128
_FREE = _PER_CORE // _P             # 32_768 f32 per partition per tensor
_WIDTHS = [4096] * 7 + [2048, 1024, 512, 512]   # sums to 32768
_NT = len(_WIDTHS)                  # 11 tiles
_OFFS = [sum(_WIDTHS[:i]) for i in range(_NT)]
_NBUF = 4
_WMAX = 4096
_DIVISOR = float(_TOTAL // 4)       # 8_388_608  (elements per subband)

_CACHE = {}


def _build_nc():
    from contextlib import ExitStack
    import concourse.bass as bass
    import concourse.mybir as mybir

    f32 = mybir.dt.float32
    nc = bass.Bass("TRN2", target_bir_lowering=False,
                   enable_partition_id=False)
    xy = nc.dram_tensor("xy", [_P, 2, _FREE], f32, kind="ExternalInput")
    out = nc.dram_tensor("out", [_P, _NT], f32, kind="ExternalOutput")

    ctx = ExitStack()
    nc._ctx = ctx  # keep SBUF/semaphore handles alive for compile
    slots = [ctx.enter_context(nc.sbuf_tensor(f"slot{i}", [_P, 2, _WMAX], f32))
             for i in range(_NBUF)]
    stats = ctx.enter_context(nc.sbuf_tensor([_P, _NT], f32))
    zbias = ctx.enter_context(nc.sbuf_tensor([_P, 1], f32))
    qsem_s = ctx.enter_context(nc.semaphore())   # Sync-queue completions
    qsem_a = ctx.enter_context(nc.semaphore())   # Scalar-queue completions
    dve_sem = ctx.enter_context(nc.semaphore())
    act_sem = ctx.enter_context(nc.semaphore())
    block = ctx.enter_context(nc.Block())

    def halves(t):
        # (dram slice, sbuf slice) for each queue half of tile t
        w, o = _WIDTHS[t], _OFFS[t]
        h = w // 2
        st = slots[t % _NBUF]
        return (
            (xy[:, :, o:o + h], st[:, :, 0:h]),
            (xy[:, :, o + h:o + w], st[:, :, h:w]),
        )

    @block.sync
    def _(sync):
        for t in range(_NT):
            if t >= _NBUF:
                # slot free once ACT (last reader) finished tile t-NBUF
                sync.wait_ge(act_sem, t - _NBUF + 1)
            (src, dst), _h2 = halves(t)
            sync.dma_start(out=dst, in_=src).then_inc(qsem_s, 16)
        # hold the door until both queues fully drained
        sync.wait_ge(qsem_s, 16 * _NT)
        sync.wait_ge(qsem_a, 16 * (_NT + 1))   # ACT's halves + stats store

    @block.scalar
    def _(scalar):
        done = 0  # squares emitted so far

        def emit_dma(t):
            _h1, (src, dst) = halves(t)
            scalar.dma_start(out=dst, in_=src).then_inc(qsem_a, 16)

        def emit_sq(t):
            nonlocal done
            w = _WIDTHS[t]
            st = slots[t % _NBUF]
            scalar.wait_ge(dve_sem, t + 2)
            scalar.activation(
                st[:, 0, :w], st[:, 0, :w],
                mybir.ActivationFunctionType.Square,
                bias=zbias[:, 0:1], accum_out=stats[:, t:t + 1],
            ).then_inc(act_sem, 1)
            done += 1

        for t in range(_NT):
            # program order gives the slot-reuse guarantee for ACT-issued
            # halves: square(t-NBUF) must precede dma(t)
            while t >= _NBUF and done < t - _NBUF + 1:
                emit_sq(done)
            emit_dma(t)
        while done < _NT:
            emit_sq(done)
        scalar.dma_start(out=out[:], in_=stats[:]).then_inc(qsem_a, 16)

    @block.vector
    def _(vector):
        vector.memset(zbias[:], 0.0).then_inc(dve_sem, 1)
        for t in range(_NT):
            w = _WIDTHS[t]
            st = slots[t % _NBUF]
            vector.wait_ge(qsem_s, 16 * (t + 1))
            vector.wait_ge(qsem_a, 16 * (t + 1))
            vector.tensor_sub(st[:, 0, :w], st[:, 0, :w], st[:, 1, :w]) \
                  .then_inc(dve_sem, 1)

    ctx.close()
    return nc


def _run(in_maps, trace=False):
    from concourse.bass_utils import run_bass_kernel_spmd

    if "nc" not in _CACHE:
        _CACHE["nc"] = _build_nc()
    return run_bass_kernel_spmd(
        _CACHE["nc"], in_maps, list(range(_NCORES)), trace=trace
    )


def _make_in_maps(input, target):
    xs = np.asarray(input, dtype=np.float32).reshape(_NCORES, _P, _FREE)
    ys = np.asarray(target, dtype=np.float32).reshape(_NCORES, _P, _FREE)
    maps = []
    for c in range(_NCORES):
        xy = np.empty((_P, 2, _FREE), dtype=np.float32)
        xy[:, 0, :] = xs[c]
        xy[:, 1, :] = ys[c]
        maps.append({"xy": xy})
    return maps


def _finish(results):
    total = 0.0
    for r in results:
        total += r["out"].astype(np.float64).sum()
    return np.array(total / _DIVISOR, dtype=np.float32)


def kernel(input, target):
    res = _run(_make_in_maps(input, target), trace=False)
    return _finish(res.results)


# revision 3
# speedup vs baseline: 1.0132x; 1.0132x over previous
"""HaarMSELoss kernel for Trainium2 (8 NeuronCores, data-parallel).

Math: the 2x2 Haar transform used by the reference is (up to the 0.5
scaling) an orthogonal Hadamard transform, so for each 2x2 block
LL^2+LH^2+HL^2+HH^2 == a^2+b^2+c^2+d^2 of the block entries of
(input - target).  Hence

  loss = sum_bands mean((haar(x)-haar(y))^2)
       = sum((x-y)^2) / (B*C*(H/2)*(W/2))

i.e. a pure squared-difference reduction.  Each core reduces 1/8 of the
elements; the host sums the 8x128 per-partition partials (f64) and
divides.

Layout: per core the two chunks are interleaved host-side into one
[128, 2, FREE] array (row p = x-row p, y-row p) so each SBUF tile of
both operands arrives with a single dma_start.

Per-core DMA bandwidth is capped ~400 GB/s (HBM domain share / SBUF AXI
fabric) and a single HWDGE dma_start already fans out across all 16
SDMA engines, so one queue saturates it; the streaming phase is the
roofline.  Tile widths taper at the end (4096 x7, 2048, 1024, 512, 512)
so the serial sub+square tail after the last byte lands is minimal.

Raw bass pipeline (explicit sems; every wait is a single-sem wait):
  SP  : dma loads (slot-recycled against ACT), final stats store
  DVE : d = x - y in place
  ACT : stats[:,t] = sum(d^2) via activation(Square, accum_out)
"""

import numpy as np

_B, _C, _H, _W = 4, 32, 512, 512
_TOTAL = _B * _C * _H * _W          # 33_554_432
_NCORES = 8
_PER_CORE = _TOTAL // _NCORES       # 4_194_304
_P = 128
_FREE = _PER_CORE // _P             # 32_768 f32 per partition per tensor
_WIDTHS = [4096] * 7 + [2048, 1024, 512, 512]   # sums to 32768
_NT = len(_WIDTHS)                  # 11 tiles
_OFFS = [sum(_WIDTHS[:i]) for i in range(_NT)]
_NBUF = 4
_WMAX = 4096
_DIVISOR = float(_TOTAL // 4)       # 8_388_608  (elements per subband)

_CACHE = {}


def _build_nc():
    from contextlib import ExitStack
    import concourse.bass as bass
    import concourse.mybir as mybir

    f32 = mybir.dt.float32
    nc = bass.Bass("TRN2", target_bir_lowering=False)
    xy = nc.dram_tensor("xy", [_P, 2, _FREE], f32, kind="ExternalInput")
    out = nc.dram_tensor("out", [_P, _NT], f32, kind="ExternalOutput")

    ctx = ExitStack()
    nc._ctx = ctx  # keep SBUF/semaphore handles alive for compile
    slots = [ctx.enter_context(nc.sbuf_tensor(f"slot{i}", [_P, 2, _WMAX], f32))
             for i in range(_NBUF)]
    stats = ctx.enter_context(nc.sbuf_tensor([_P, _NT], f32))
    zbias = ctx.enter_context(nc.sbuf_tensor([_P, 1], f32))
    dma_sem = ctx.enter_context(nc.semaphore())
    dve_sem = ctx.enter_context(nc.semaphore())
    act_sem = ctx.enter_context(nc.semaphore())
    block = ctx.enter_context(nc.Block())

    @block.sync
    def _(sync):
        for t in range(_NT):
            if t >= _NBUF:
                # slot free once ACT (last reader) finished tile t-NBUF
                sync.wait_ge(act_sem, t - _NBUF + 1)
            w, o = _WIDTHS[t], _OFFS[t]
            st = slots[t % _NBUF]
            sync.dma_start(
                out=st[:, :, :w], in_=xy[:, :, o:o + w]
            ).then_inc(dma_sem, 16)
        # act_sem increments fire on ACTIVATION_READ_ACCUMULATOR complete,
        # so stats is fully written before the store is generated
        sync.wait_ge(act_sem, _NT)
        sync.dma_start(out=out[:], in_=stats[:]).then_inc(dma_sem, 16)
        sync.wait_ge(dma_sem, 16 * (_NT + 1))  # store landed

    @block.vector
    def _(vector):
        vector.memset(zbias[:], 0.0).then_inc(dve_sem, 1)
        for t in range(_NT):
            w = _WIDTHS[t]
            st = slots[t % _NBUF]
            vector.wait_ge(dma_sem, 16 * (t + 1))
            vector.tensor_sub(st[:, 0, :w], st[:, 0, :w], st[:, 1, :w]) \
                  .then_inc(dve_sem, 1)

    @block.scalar
    def _(scalar):
        for t in range(_NT):
            w = _WIDTHS[t]
            st = slots[t % _NBUF]
            scalar.wait_ge(dve_sem, t + 2)
            scalar.activation(
                st[:, 0, :w], st[:, 0, :w],
                mybir.ActivationFunctionType.Square,
                bias=zbias[:, 0:1], accum_out=stats[:, t:t + 1],
            ).then_inc(act_sem, 1)

    ctx.close()
    return nc


def _run(in_maps, trace=False):
    from concourse.bass_utils import run_bass_kernel_spmd

    if "nc" not in _CACHE:
        _CACHE["nc"] = _build_nc()
    return run_bass_kernel_spmd(
        _CACHE["nc"], in_maps, list(range(_NCORES)), trace=trace
    )


def _make_in_maps(input, target):
    xs = np.asarray(input, dtype=np.float32).reshape(_NCORES, _P, _FREE)
    ys = np.asarray(target, dtype=np.float32).reshape(_NCORES, _P, _FREE)
    maps = []
    for c in range(_NCORES):
        xy = np.empty((_P, 2, _FREE), dtype=np.float32)
        xy[:, 0, :] = xs[c]
        xy[:, 1, :] = ys[c]
        maps.append({"xy": xy})
    return maps


def _finish(results):
    total = 0.0
    for r in results:
        total += r["out"].astype(np.float64).sum()
    return np.array(total / _DIVISOR, dtype=np.float32)


def kernel(input, target):
    res = _run(_make_in_maps(input, target), trace=False)
    return _finish(res.results)
